# revision 1
# baseline (speedup 1.0000x reference)
"""nn_MatchingModule kernel for 8 trn2 NeuronCores.

Data-parallel over batch (B=8 -> one batch element per core), exactly as the
sharding hint suggests: warp, correlation and the three convs are local in
batch, so there is no cross-device communication. The whole pipeline is one
jitted XLA program replicated on the 8 cores via pmap; weights are broadcast.

Hardcoded problem shape: B=8, C=128, H=W=128; flow [8,2,64,64];
w1[64,49,3,3] b1[64], w2[32,64,3,3] b2[32], w3[2,32,5,5] b3[2].
"""

import numpy as np
import jax
import jax.numpy as jnp
from jax import lax

WARP_WEIGHT = 2.5
MD = 3
NEG_SLOPE = 0.1
H = W = 128


def _upsample_matrix(n_in: int) -> np.ndarray:
    """Exact bilinear 2x upsample (align_corners=False) as a matrix [2n, n]."""
    n_out = 2 * n_in
    U = np.zeros((n_out, n_in), np.float32)
    for i in range(n_out):
        # source position (i + 0.5)/2 - 0.5 = i/2 - 0.25
        lo = i // 2 - 1 if i % 2 == 0 else i // 2
        hi = lo + 1
        w_hi = 0.75 if i % 2 == 0 else 0.25
        lo_c = min(max(lo, 0), n_in - 1)
        hi_c = min(max(hi, 0), n_in - 1)
        U[i, lo_c] += 1.0 - w_hi
        U[i, hi_c] += w_hi
    return U


_UY = _upsample_matrix(64)  # [128, 64]


def _pipeline(f1, f2, fl, w1, b1, w2, b2, w3, b3):
    """Per-core: f1,f2 [C,H,W]; fl [2,64,64] -> out [2,H,W]."""
    C = f1.shape[0]
    U = jnp.asarray(_UY)
    # bilinear 2x upsample as two small matmuls (exact, verified vs jax.image)
    flow_up = jnp.einsum('yk,ckl,xl->cyx', U, fl, U)          # [2,128,128]

    d = flow_up * WARP_WEIGHT
    yy, xx = jnp.meshgrid(jnp.arange(H, dtype=jnp.float32),
                          jnp.arange(W, dtype=jnp.float32), indexing='ij')
    x = xx + d[0]
    y = yy + d[1]
    x0f, y0f = jnp.floor(x), jnp.floor(y)
    wx, wy = x - x0f, y - y0f
    x0 = x0f.astype(jnp.int32)
    y0 = y0f.astype(jnp.int32)

    f2flat = f2.reshape(C, H * W)

    def gather(yi, xi):
        valid = ((yi >= 0) & (yi < H) & (xi >= 0) & (xi < W)).astype(jnp.float32)
        yc = jnp.clip(yi, 0, H - 1)
        xc = jnp.clip(xi, 0, W - 1)
        v = jnp.take(f2flat, (yc * W + xc).reshape(-1), axis=1).reshape(C, H, W)
        return v * valid[None]

    f2w = (gather(y0, x0) * ((1 - wx) * (1 - wy))[None]
           + gather(y0, x0 + 1) * (wx * (1 - wy))[None]
           + gather(y0 + 1, x0) * ((1 - wx) * wy)[None]
           + gather(y0 + 1, x0 + 1) * (wx * wy)[None])

    # windowed cost volume: [49, H, W]
    f2p = jnp.pad(f2w, ((0, 0), (MD, MD), (MD, MD)))
    outs = [jnp.mean(f1 * lax.dynamic_slice(f2p, (0, dy, dx), (C, H, W)), axis=0)
            for dy in range(2 * MD + 1) for dx in range(2 * MD + 1)]
    corr = jnp.stack(outs, axis=0)

    def conv(xin, w, b, pad):
        yv = lax.conv_general_dilated(
            xin[None], w, window_strides=(1, 1),
            padding=[(pad, pad), (pad, pad)],
            dimension_numbers=('NCHW', 'OIHW', 'NCHW'))[0]
        return yv + b[:, None, None]

    h = conv(corr, w1, b1, 1)
    h = jnp.where(h >= 0, h, NEG_SLOPE * h)
    h = conv(h, w2, b2, 1)
    h = jnp.where(h >= 0, h, NEG_SLOPE * h)
    h = conv(h, w3, b3, 2)
    return flow_up + h


_PFN = None


def _get_pfn():
    global _PFN
    if _PFN is None:
        devs = jax.devices()[:8]
        _PFN = jax.pmap(
            _pipeline, devices=devs,
            in_axes=(0, 0, 0, None, None, None, None, None, None))
    return _PFN


def kernel(features1, features2, flow, w1, b1, w2, b2, w3, b3):
    pfn = _get_pfn()
    out = pfn(jnp.asarray(features1), jnp.asarray(features2), jnp.asarray(flow),
              jnp.asarray(w1), jnp.asarray(b1), jnp.asarray(w2),
              jnp.asarray(b2), jnp.asarray(w3), jnp.asarray(b3))
    out.block_until_ready()
    return np.asarray(out).astype(np.float32)


# revision 2
# speedup vs baseline: 1.0004x; 1.0004x over previous
"""nn_MatchingModule kernel for 8 trn2 NeuronCores.

Data-parallel over batch (B=8 -> one batch element per core), exactly as the
sharding hint suggests: warp, correlation and the three convs are local in
batch, so there is no cross-device communication. The whole pipeline is one
jitted XLA program replicated on the 8 cores via pmap; weights are broadcast.

Hardcoded problem shape: B=8, C=128, H=W=128; flow [8,2,64,64];
w1[64,49,3,3] b1[64], w2[32,64,3,3] b2[32], w3[2,32,5,5] b3[2].
"""

import numpy as np
import jax
import jax.numpy as jnp
from jax import lax

WARP_WEIGHT = 2.5
MD = 3
NEG_SLOPE = 0.1
H = W = 128


def _upsample_matrix(n_in: int) -> np.ndarray:
    """Exact bilinear 2x upsample (align_corners=False) as a matrix [2n, n]."""
    n_out = 2 * n_in
    U = np.zeros((n_out, n_in), np.float32)
    for i in range(n_out):
        # source position (i + 0.5)/2 - 0.5 = i/2 - 0.25
        lo = i // 2 - 1 if i % 2 == 0 else i // 2
        hi = lo + 1
        w_hi = 0.75 if i % 2 == 0 else 0.25
        lo_c = min(max(lo, 0), n_in - 1)
        hi_c = min(max(hi, 0), n_in - 1)
        U[i, lo_c] += 1.0 - w_hi
        U[i, hi_c] += w_hi
    return U


_UY = _upsample_matrix(64)  # [128, 64]


def _pipeline(f1, f2, fl, w1, b1, w2, b2, w3, b3):
    """Per-core: f1,f2 [C,H,W]; fl [2,64,64] -> out [2,H,W]."""
    C = f1.shape[0]
    U = jnp.asarray(_UY)
    # bilinear 2x upsample as two small matmuls (exact, verified vs jax.image)
    flow_up = jnp.einsum('yk,ckl,xl->cyx', U, fl, U)          # [2,128,128]

    d = flow_up * WARP_WEIGHT
    yy, xx = jnp.meshgrid(jnp.arange(H, dtype=jnp.float32),
                          jnp.arange(W, dtype=jnp.float32), indexing='ij')
    x = xx + d[0]
    y = yy + d[1]
    x0f, y0f = jnp.floor(x), jnp.floor(y)
    wx, wy = x - x0f, y - y0f
    x0 = x0f.astype(jnp.int32)
    y0 = y0f.astype(jnp.int32)

    # bf16 on the heavy data path (gather + correlation + convs), fp32
    # accumulation. Simulated end-to-end error: resid_var ~3e-9.
    f2flat = f2.reshape(C, H * W).astype(jnp.bfloat16)

    def gather(yi, xi):
        valid = ((yi >= 0) & (yi < H) & (xi >= 0) & (xi < W)).astype(jnp.float32)
        yc = jnp.clip(yi, 0, H - 1)
        xc = jnp.clip(xi, 0, W - 1)
        v = jnp.take(f2flat, (yc * W + xc).reshape(-1), axis=1).reshape(C, H, W)
        return v.astype(jnp.float32) * valid[None]

    f2w = (gather(y0, x0) * ((1 - wx) * (1 - wy))[None]
           + gather(y0, x0 + 1) * (wx * (1 - wy))[None]
           + gather(y0 + 1, x0) * ((1 - wx) * wy)[None]
           + gather(y0 + 1, x0 + 1) * (wx * wy)[None])

    # windowed cost volume: [49, H, W]; bf16 products, fp32 reduction
    f1b = f1.astype(jnp.bfloat16)
    f2p = jnp.pad(f2w.astype(jnp.bfloat16), ((0, 0), (MD, MD), (MD, MD)))
    outs = [jnp.mean(f1b * lax.dynamic_slice(f2p, (0, dy, dx), (C, H, W)),
                     axis=0, dtype=jnp.float32)
            for dy in range(2 * MD + 1) for dx in range(2 * MD + 1)]
    corr = jnp.stack(outs, axis=0)

    def conv(xin, w, b, pad):
        yv = lax.conv_general_dilated(
            xin[None].astype(jnp.bfloat16), w.astype(jnp.bfloat16),
            window_strides=(1, 1), padding=[(pad, pad), (pad, pad)],
            dimension_numbers=('NCHW', 'OIHW', 'NCHW'),
            preferred_element_type=jnp.float32)[0]
        return yv + b[:, None, None]

    h = conv(corr, w1, b1, 1)
    h = jnp.where(h >= 0, h, NEG_SLOPE * h)
    h = conv(h, w2, b2, 1)
    h = jnp.where(h >= 0, h, NEG_SLOPE * h)
    h = conv(h, w3, b3, 2)
    return flow_up + h


_PFN = None


def _get_pfn():
    global _PFN
    if _PFN is None:
        devs = jax.devices()[:8]
        _PFN = jax.pmap(
            _pipeline, devices=devs,
            in_axes=(0, 0, 0, None, None, None, None, None, None))
    return _PFN


def kernel(features1, features2, flow, w1, b1, w2, b2, w3, b3):
    pfn = _get_pfn()
    out = pfn(jnp.asarray(features1), jnp.asarray(features2), jnp.asarray(flow),
              jnp.asarray(w1), jnp.asarray(b1), jnp.asarray(w2),
              jnp.asarray(b2), jnp.asarray(w3), jnp.asarray(b3))
    out.block_until_ready()
    return np.asarray(out).astype(np.float32)


# revision 3
# speedup vs baseline: 1.0103x; 1.0100x over previous
"""nn_MatchingModule kernel for 8 trn2 NeuronCores.

Data-parallel over batch (B=8 -> one batch element per core), exactly as the
sharding hint suggests: warp, correlation and the three convs are local in
batch, so there is no cross-device communication. The whole pipeline is one
jitted XLA program replicated on the 8 cores via pmap; weights are broadcast.

Hardcoded problem shape: B=8, C=128, H=W=128; flow [8,2,64,64];
w1[64,49,3,3] b1[64], w2[32,64,3,3] b2[32], w3[2,32,5,5] b3[2].
"""

import numpy as np
import jax
import jax.numpy as jnp
from jax import lax

WARP_WEIGHT = 2.5
MD = 3
NEG_SLOPE = 0.1
H = W = 128


def _upsample_matrix(n_in: int) -> np.ndarray:
    """Exact bilinear 2x upsample (align_corners=False) as a matrix [2n, n]."""
    n_out = 2 * n_in
    U = np.zeros((n_out, n_in), np.float32)
    for i in range(n_out):
        # source position (i + 0.5)/2 - 0.5 = i/2 - 0.25
        lo = i // 2 - 1 if i % 2 == 0 else i // 2
        hi = lo + 1
        w_hi = 0.75 if i % 2 == 0 else 0.25
        lo_c = min(max(lo, 0), n_in - 1)
        hi_c = min(max(hi, 0), n_in - 1)
        U[i, lo_c] += 1.0 - w_hi
        U[i, hi_c] += w_hi
    return U


_UY = _upsample_matrix(64)  # [128, 64]


def _pipeline(f1, f2, fl, w1, b1, w2, b2, w3, b3):
    """Per-core: f1,f2 [C,H,W]; fl [2,64,64] -> out [2,H,W]."""
    C = f1.shape[0]
    U = jnp.asarray(_UY)
    # bilinear 2x upsample as two small matmuls (exact, verified vs jax.image)
    flow_up = jnp.einsum('yk,ckl,xl->cyx', U, fl, U)          # [2,128,128]

    d = flow_up * WARP_WEIGHT
    yy, xx = jnp.meshgrid(jnp.arange(H, dtype=jnp.float32),
                          jnp.arange(W, dtype=jnp.float32), indexing='ij')
    x = xx + d[0]
    y = yy + d[1]
    x0f, y0f = jnp.floor(x), jnp.floor(y)
    wx, wy = x - x0f, y - y0f
    x0 = x0f.astype(jnp.int32)
    y0 = y0f.astype(jnp.int32)

    # bf16 on the heavy data path (gather + correlation + convs), fp32
    # accumulation. Simulated end-to-end error: resid_var ~3e-9.
    f2flat = f2.reshape(C, H * W).astype(jnp.bfloat16)

    def gather(yi, xi):
        valid = ((yi >= 0) & (yi < H) & (xi >= 0) & (xi < W)).astype(jnp.float32)
        yc = jnp.clip(yi, 0, H - 1)
        xc = jnp.clip(xi, 0, W - 1)
        v = jnp.take(f2flat, (yc * W + xc).reshape(-1), axis=1).reshape(C, H, W)
        return v.astype(jnp.float32) * valid[None]

    f2w = (gather(y0, x0) * ((1 - wx) * (1 - wy))[None]
           + gather(y0, x0 + 1) * (wx * (1 - wy))[None]
           + gather(y0 + 1, x0) * ((1 - wx) * wy)[None]
           + gather(y0 + 1, x0 + 1) * (wx * wy)[None])

    # windowed cost volume via per-row batched matmuls on the PE:
    # G_dy[y, x, s] = sum_c f1[c,y,x] * f2p[c, y+dy, s], then the 7 needed
    # dx-diagonals are extracted with a cheap gather. Replaces 49 large
    # DVE-bound elementwise products with 7 batched GEMMs.
    f1b = f1.astype(jnp.bfloat16)
    f2p = jnp.pad(f2w.astype(jnp.bfloat16), ((0, 0), (MD, MD), (MD, MD)))
    xidx = jnp.arange(W)[:, None] + jnp.arange(2 * MD + 1)[None, :]   # [W,7]
    gidx = jnp.broadcast_to(xidx[None], (H, W, 2 * MD + 1))
    douts = []
    for dy in range(2 * MD + 1):
        rows = lax.dynamic_slice(f2p, (0, dy, 0), (C, H, W + 2 * MD))
        G = jnp.einsum('cyx,cys->yxs', f1b, rows,
                       preferred_element_type=jnp.float32)            # [H,W,W+6]
        douts.append(jnp.take_along_axis(G, gidx, axis=2))            # [H,W,7]
    corr = (jnp.stack(douts, 0).transpose(0, 3, 1, 2).reshape(49, H, W)
            / np.float32(C))

    def conv(xin, w, b, pad):
        yv = lax.conv_general_dilated(
            xin[None].astype(jnp.bfloat16), w.astype(jnp.bfloat16),
            window_strides=(1, 1), padding=[(pad, pad), (pad, pad)],
            dimension_numbers=('NCHW', 'OIHW', 'NCHW'),
            preferred_element_type=jnp.float32)[0]
        return yv + b[:, None, None]

    h = conv(corr, w1, b1, 1)
    h = jnp.where(h >= 0, h, NEG_SLOPE * h)
    h = conv(h, w2, b2, 1)
    h = jnp.where(h >= 0, h, NEG_SLOPE * h)
    h = conv(h, w3, b3, 2)
    return flow_up + h


_PFN = None


def _get_pfn():
    global _PFN
    if _PFN is None:
        devs = jax.devices()[:8]
        _PFN = jax.pmap(
            _pipeline, devices=devs,
            in_axes=(0, 0, 0, None, None, None, None, None, None))
    return _PFN


def kernel(features1, features2, flow, w1, b1, w2, b2, w3, b3):
    pfn = _get_pfn()
    out = pfn(jnp.asarray(features1), jnp.asarray(features2), jnp.asarray(flow),
              jnp.asarray(w1), jnp.asarray(b1), jnp.asarray(w2),
              jnp.asarray(b2), jnp.asarray(w3), jnp.asarray(b3))
    out.block_until_ready()
    return np.asarray(out).astype(np.float32)
